# revision 2
# baseline (speedup 1.0000x reference)
"""Trainium2 Bass kernel v2 for nn_LSTMActor.

vs baseline:
- trunk: b_trunk folded into a padded contraction row (kills LN add); obsT on
  gpsimd queue, wtr alternating sync/scalar queues; LSTM weights prefetched
  right after the trunk DMA stream.
- LSTM: hybrid gates GEMM: k-tiles 0..3 bf16 (W x512), k-tiles 4..7 as 2 fp8
  DoubleRow pairs (W x32 fp8, h x16 fp8 -> x512). preT stored x512; Pool
  preloads it into PSUM; matmuls accumulate (start=False). Activations batched
  per j-pair from PSUM with fused 1/512 scale; elementwise split DVE/Pool.
- Head: W1 bf16 per step off bf16 h; W2 transposed (out [6,256] per step,
  bias+tanh fused on ACT into muT).
"""

import numpy as np
import ml_dtypes

import concourse.bass as bass
import concourse.tile as tile
from concourse import mybir, bacc
from concourse import bass_utils
from concourse.masks import make_identity

BF = ml_dtypes.bfloat16
E4 = ml_dtypes.float8_e4m3
F32 = mybir.dt.float32
BF16 = mybir.dt.bfloat16
FP8 = mybir.dt.float8e4
DR = mybir.MatmulPerfMode.DoubleRow

B, R, Fd, H, A, T = 2048, 39200, 1024, 1024, 6, 16
NC_ = 8
BS = B // NC_          # 256 rows per core
NB = BS // 128         # 2 b-tiles per core
KT = 128
RP = ((R + KT - 1) // KT) * KT + 128   # 39424: pad + bias k-tile
NK = RP // KT          # 308 trunk K-tiles
KH = H // 128          # 8 k-tiles for H-dim GEMMs
KBF = 4                # gates k-tiles 0..3 in bf16
M4 = 4 * H // 128      # 32 m-tiles of gates
H2 = H // 2
KG = 2                 # trunk K-tiles per DMA batch
SCL = 512.0            # gates psum scale
SH, SW8 = 16.0, 32.0   # h-fp8 scale, W-fp8 scale (SH*SW8 == SCL)

_CACHE = {}


def _build():
    nc = bacc.Bacc("TRN2", target_bir_lowering=False, debug=False)

    obsT_d = nc.dram_tensor("obsT", [RP, BS], BF16, kind="ExternalInput")
    wtr_d = nc.dram_tensor("wtr", [RP, Fd], BF16, kind="ExternalInput")
    wih_d = nc.dram_tensor("wih", [M4, 128, KH * 128], BF16, kind="ExternalInput")
    whhbf_d = nc.dram_tensor("whhbf", [128, KBF, 4 * H], BF16, kind="ExternalInput")
    wf8_d = nc.dram_tensor("wf8", [128, 2 * M4 * 2 * 128], FP8, kind="ExternalInput")
    w1_d = nc.dram_tensor("w1", [H, H2], BF16, kind="ExternalInput")
    w2_d = nc.dram_tensor("w2", [H2, A], BF16, kind="ExternalInput")
    gam_d = nc.dram_tensor("gam", [Fd], BF16, kind="ExternalInput")
    bet_d = nc.dram_tensor("bet", [Fd], BF16, kind="ExternalInput")
    bsum_d = nc.dram_tensor("bsum", [128, M4], F32, kind="ExternalInput")  # x512
    b1_d = nc.dram_tensor("b1", [128, H2 // 128], F32, kind="ExternalInput")
    b2_d = nc.dram_tensor("b2", [A, 1], F32, kind="ExternalInput")
    mu_d = nc.dram_tensor("mu", [A, T * BS], F32, kind="ExternalOutput")

    AF = mybir.ActivationFunctionType
    ALU = mybir.AluOpType

    def bc(ap1d, p=128):
        return bass.AP(tensor=ap1d.tensor, offset=ap1d.offset,
                       ap=[[0, p]] + [list(x) for x in ap1d.ap])

    with tile.TileContext(nc) as tc:
        with (
            tc.tile_pool(name="const", bufs=1) as cst,
            tc.tile_pool(name="state", bufs=1) as st,
            tc.tile_pool(name="wstream", bufs=2) as ws,
            tc.tile_pool(name="work", bufs=1) as wk,
            tc.tile_pool(name="acts", bufs=2) as ac,
        ):
            # ---- resident constants ----
            ident = cst.tile([128, 128], BF16)
            make_identity(nc, ident)
            whhbf = cst.tile([128, KBF, 4 * H], BF16)     # 32KB/part (x512)
            wf8 = cst.tile([128, 2, M4, 2, 128], FP8)     # 16KB/part (x32)
            w1_sb = cst.tile([128, KH, H2], BF16)         # 8KB/part
            w2_sb = cst.tile([128, H2 // 128, A], BF16)
            gam_b = cst.tile([128, Fd], BF16)
            bet_b = cst.tile([128, Fd], BF16)
            bsum_sb = cst.tile([128, M4], F32)            # x512 host-side
            b1_sb = cst.tile([128, H2 // 128], F32)
            b2_sb = cst.tile([A, 1], F32)
            eps_t = cst.tile([128, 1], F32)
            nc.vector.memset(eps_t, 1e-5)

            # ---- persistent state ----
            xT = st.tile([128, KH, BS], BF16)
            preT = st.tile([128, 4, 4, 2, BS], BF16)      # x512 [p,jj,gate,j01,b]
            c_st = st.tile([128, KH, BS], F32)
            h_bf = [st.tile([128, KH, BS], BF16, name=f"hbf{i}") for i in range(2)]
            h8 = [st.tile([128, 2, 2, BS], FP8, name=f"h8_{i}") for i in range(2)]
            relu1 = [st.tile([128, H2 // 128, BS], BF16, name=f"r1_{i}")
                     for i in range(2)]
            muT = st.tile([A, T, BS], F32)

            wtr_r = wtr_d.ap().rearrange("(ko p) n -> p ko n", p=128)
            obsT_r = obsT_d.ap().rearrange("(ko p) b -> p ko b", p=128)

            # ================= Phase 1: trunk GEMM =================
            front = tc.tile_pool(name="front", bufs=1)
            fr = front.__enter__()
            wihA = fr.tile([128, 8, KH * 128], BF16)      # 16KB/part, m 0..7
            with tc.tile_pool(name="ps_trunk", bufs=1, space="PSUM") as pst:
                psx = pst.tile([128, NB, Fd], F32)        # 8 banks
                for kg in range(0, NK, KG):
                    kn = min(KG, NK - kg)
                    wt = ws.tile([128, KG, Fd], BF16, tag="wtr", bufs=6)
                    ot = ws.tile([128, KG, BS], BF16, tag="obsT", bufs=4)
                    nc.sync.dma_start(wt[:, :kn, :], wtr_r[:, kg : kg + kn, :])
                    nc.sync.dma_start(ot[:, :kn, :], obsT_r[:, kg : kg + kn, :])
                    if kg == 0:
                        # small consts early (needed right at LN)
                        nc.sync.dma_start(gam_b, bc(gam_d.ap()))
                        nc.sync.dma_start(bet_b, bc(bet_d.ap()))
                        nc.sync.dma_start(bsum_sb, bsum_d.ap())
                        nc.sync.dma_start(b1_sb, b1_d.ap())
                        nc.sync.dma_start(b2_sb, b2_d.ap())
                    for kk in range(kn):
                        k = kg + kk
                        for b in range(NB):
                            lhsT = ot[:, kk, b * 128 : (b + 1) * 128]
                            for n in range(2):
                                nc.tensor.matmul(
                                    psx[:, b, n * 512 : (n + 1) * 512],
                                    lhsT,
                                    wt[:, kk, n * 512 : (n + 1) * 512],
                                    start=(k == 0),
                                    stop=(k == NK - 1),
                                )
                # weight prefetch behind the trunk stream on the same queue
                for m in range(8):
                    nc.sync.dma_start(wihA[:, m, :], wih_d.ap()[m])
                nc.sync.dma_start(whhbf, whhbf_d.ap())
                nc.sync.dma_start(
                    wf8.rearrange("p a b c d -> p (a b c d)"), wf8_d.ap())
                nc.sync.dma_start(
                    w1_sb, w1_d.ap().rearrange("(ko p) n -> p ko n", p=128))
                nc.sync.dma_start(
                    w2_sb, w2_d.ap().rearrange("(ko p) n -> p ko n", p=128))

                # ============ Phase 2: LayerNorm + tanh ============
                xa = fr.tile([128, NB, Fd], BF16, tag="xa")
                for b in range(NB):
                    stats = wk.tile([128, 2, 6], F32, tag="stats")
                    for s in range(2):
                        nc.vector.bn_stats(
                            out=stats[:, s, :],
                            in_=psx[:, b, s * 512 : (s + 1) * 512])
                    mv = wk.tile([128, 2], F32, tag="mv")
                    nc.vector.bn_aggr(out=mv, in_=stats)
                    rstd = wk.tile([128, 1], F32, tag="rstd")
                    nc.scalar.activation(
                        out=rstd, in_=mv[:, 1:2], func=AF.Sqrt, bias=eps_t,
                        scale=1.0)
                    nc.vector.reciprocal(out=rstd, in_=rstd)
                    xs = wk.tile([128, Fd], F32, tag="xs", bufs=1)
                    nc.vector.scalar_tensor_tensor(
                        out=xs, in0=psx[:, b, :], scalar=mv[:, 0:1], in1=gam_b,
                        op0=ALU.subtract, op1=ALU.mult)
                    nc.vector.scalar_tensor_tensor(
                        out=xs, in0=xs, scalar=rstd, in1=bet_b,
                        op0=ALU.mult, op1=ALU.add)
                    nc.scalar.activation(out=xa[:, b, :], in_=xs, func=AF.Tanh)

            # ============ Phase 3: transpose x -> xT ============
            with tc.tile_pool(name="ps_tr", bufs=4, space="PSUM") as ptr:
                for b in range(NB):
                    for f in range(KH):
                        pt = ptr.tile([128, 128], BF16, tag="tr")
                        nc.tensor.transpose(
                            pt, xa[:, b, f * 128 : (f + 1) * 128], ident)
                        nc.scalar.activation(
                            out=xT[:, f, b * 128 : (b + 1) * 128], in_=pt,
                            func=AF.Copy)

            # ============ Phase 4: preT = 512*(W_ih^T x^T) + 512*bias ======
            with tc.tile_pool(name="ps_pre", bufs=4, space="PSUM") as ppr:
                for m in range(M4):
                    if m < 8:
                        wm = wihA[:, m, :].rearrange("p (k j) -> p k j", j=128)
                    else:
                        wmt = ws.tile([128, KH, 128], BF16, tag="wih", bufs=4)
                        nc.sync.dma_start(
                            wmt, wih_d.ap()[m].rearrange("p (k j) -> p k j", j=128))
                        wm = wmt
                    ps = ppr.tile([128, BS], F32, tag="pre")
                    for k in range(KH):
                        nc.tensor.matmul(
                            ps, wm[:, k, :], xT[:, k, :],
                            start=(k == 0), stop=(k == KH - 1))
                    g, j = m // 8, m % 8
                    nc.scalar.activation(
                        out=preT[:, j // 2, g, j % 2, :], in_=ps, func=AF.Identity,
                        bias=bsum_sb[:, m : m + 1], scale=SCL)

            front.__exit__(None, None, None)

            # ============ Phase 5: LSTM ============
            with tc.tile_pool(name="ps_g", bufs=2, space="PSUM") as psg:

                def elementwise(blk, jj, t, first):
                    j0 = 2 * jj
                    cur = t % 2
                    if first:
                        src = preT[:, jj]
                    else:
                        # gates = psum + preT (DVE), ACT then reads SBUF
                        src = ac.tile([128, 4, 2, BS], F32, tag="gad")
                        nc.vector.tensor_add(src, blk, preT[:, jj])
                    scl = 1.0 / SCL
                    sif = ac.tile([128, 2, 2, BS], BF16, tag="sif")
                    nc.scalar.activation(out=sif, in_=src[:, 0:2],
                                         func=AF.Sigmoid, scale=scl)
                    tg = ac.tile([128, 2, BS], BF16, tag="tg")
                    nc.scalar.activation(out=tg, in_=src[:, 2], func=AF.Tanh,
                                         scale=scl)
                    so = ac.tile([128, 2, BS], BF16, tag="so")
                    nc.scalar.activation(out=so, in_=src[:, 3], func=AF.Sigmoid,
                                         scale=scl)
                    cs = c_st[:, j0 : j0 + 2, :]
                    if first:
                        nc.vector.tensor_mul(cs, sif[:, 0], tg)
                    else:
                        t1 = ac.tile([128, 2, BS], BF16, tag="t1")
                        nc.vector.tensor_mul(t1, sif[:, 0], tg)
                        nc.vector.tensor_mul(cs, cs, sif[:, 1])
                        nc.vector.tensor_add(cs, cs, t1)
                    tcn = ac.tile([128, 2, BS], BF16, tag="tc")
                    nc.scalar.activation(out=tcn, in_=cs, func=AF.Tanh)
                    nc.vector.tensor_mul(h_bf[cur][:, j0 : j0 + 2, :], so, tcn)
                    if jj >= 2:
                        nc.vector.scalar_tensor_tensor(
                            out=h8[cur][:, jj - 2], in0=so, scalar=SH, in1=tcn,
                            op0=ALU.mult, op1=ALU.mult)

                def head(t):
                    """relu1(t) = relu(W1^T h(t) + b1)."""
                    cur = t % 2
                    blk = psg.tile([128, 4, 2, BS], F32, tag="g")
                    for m2 in range(H2 // 128):
                        for k in range(KH):
                            nc.tensor.matmul(
                                blk[:, m2 // 2, m2 % 2, :],
                                w1_sb[:, k, m2 * 128 : (m2 + 1) * 128],
                                h_bf[cur][:, k, :],
                                start=(k == 0), stop=(k == KH - 1))
                    for m2 in range(H2 // 128):
                        nc.scalar.activation(
                            out=relu1[cur][:, m2, :],
                            in_=blk[:, m2 // 2, m2 % 2, :],
                            func=AF.Relu, bias=b1_sb[:, m2 : m2 + 1], scale=1.0)

                def w2_head(t):
                    """muT[:, t, :] = tanh(W2^T relu1(t) + b2)."""
                    cur = t % 2
                    blk = psg.tile([128, 4, 2, BS], F32, tag="g")
                    ps2 = blk[:A, 0, 0, :]
                    for k2 in range(H2 // 128):
                        nc.tensor.matmul(
                            ps2, w2_sb[:, k2, :], relu1[cur][:, k2, :],
                            start=(k2 == 0), stop=(k2 == H2 // 128 - 1))
                    nc.scalar.activation(
                        out=muT[:, t, :], in_=ps2, func=AF.Tanh,
                        bias=b2_sb, scale=1.0)

                # ---- step 0: gates = pre ----
                for jj in range(4):
                    elementwise(None, jj, 0, first=True)

                # ---- steps 1..15 ----
                for t in range(1, T):
                    prev = (t + 1) % 2
                    head(t - 1)
                    for jj in range(4):
                        blk = psg.tile([128, 4, 2, BS], F32, tag="g")
                        # one open accumulation group per PSUM bank: open the
                        # 4 bank-distinct regions (one per gate) with bf16,
                        # then close each with its fp8 DR pair
                        for j01 in range(2):
                            for g in range(4):
                                m = g * 8 + 2 * jj + j01
                                for k in range(KBF):
                                    nc.tensor.matmul(
                                        blk[:, g, j01, :],
                                        whhbf[:, k, m * 128 : (m + 1) * 128],
                                        h_bf[prev][:, k, :],
                                        start=(k == 0), stop=False,
                                        skip_group_check=True)
                            for g in range(4):
                                m = g * 8 + 2 * jj + j01
                                for p in range(2):
                                    nc.tensor.matmul(
                                        blk[:, g, j01, :],
                                        wf8[:, p, m, :, :],
                                        h8[prev][:, p],
                                        start=False, stop=(p == 1),
                                        perf_mode=DR,
                                        skip_group_check=True)
                        elementwise(blk, jj, t, first=False)
                    if t >= 2:
                        w2_head(t - 2)
                head(T - 1)
                w2_head(T - 2)
                w2_head(T - 1)

            # ---- write out ----
            nc.sync.dma_start(
                mu_d.ap().rearrange("a (t b) -> a t b", t=T), muT)

    nc.compile()
    return nc


def kernel(**inputs):
    obs = np.asarray(inputs["obs"], np.float32)
    W_trunk = np.asarray(inputs["W_trunk"], np.float32)
    b_trunk = np.asarray(inputs["b_trunk"], np.float32)
    gamma = np.asarray(inputs["gamma"], np.float32)
    beta = np.asarray(inputs["beta"], np.float32)
    W_ih = np.asarray(inputs["W_ih"], np.float32)
    b_ih = np.asarray(inputs["b_ih"], np.float32)
    W_hh = np.asarray(inputs["W_hh"], np.float32)
    b_hh = np.asarray(inputs["b_hh"], np.float32)
    W1 = np.asarray(inputs["W1"], np.float32)
    b1 = np.asarray(inputs["b1"], np.float32)
    W2 = np.asarray(inputs["W2"], np.float32)
    b2 = np.asarray(inputs["b2"], np.float32)
    num_actions = int(np.asarray(inputs["num_actions"]))
    assert num_actions == T, f"kernel hardcodes T={T}, got {num_actions}"
    assert obs.shape == (B, R)

    if "nc" not in _CACHE:
        _CACHE["nc"] = _build()
    nc = _CACHE["nc"]

    wtr = np.zeros((RP, Fd), BF)
    wtr[:R] = W_trunk.astype(BF)
    wtr[R] = b_trunk.astype(BF)          # bias contraction row
    wih = np.ascontiguousarray(
        W_ih.astype(BF).reshape(KH, 128, M4, 128).transpose(2, 1, 0, 3)
    ).reshape(M4, 128, KH * 128)
    whhbf = np.ascontiguousarray(
        (W_hh[: KBF * 128] * SCL).astype(BF).reshape(KBF, 128, 4 * H)
        .transpose(1, 0, 2))
    wf8 = np.ascontiguousarray(
        (W_hh[KBF * 128 :] * SW8).astype(E4)
        .reshape(2, 2, 128, M4, 128).transpose(2, 0, 3, 1, 4)).reshape(128, -1)
    w1 = W1.astype(BF)
    w2 = W2.astype(BF)
    bsum = np.ascontiguousarray(
        ((b_ih + b_hh) * SCL).astype(np.float32).reshape(M4, 128).T)
    b1_pm = np.ascontiguousarray(b1.astype(np.float32).reshape(H2 // 128, 128).T)
    b2_pm = b2.astype(np.float32).reshape(A, 1)

    in_maps = []
    for i in range(NC_):
        sh = obs[i * BS : (i + 1) * BS]           # [256, R]
        obsT = np.zeros((RP, BS), BF)
        obsT[:R] = np.ascontiguousarray(sh.T).astype(BF)
        obsT[R] = 1.0                              # bias row
        in_maps.append({
            "obsT": obsT, "wtr": wtr, "wih": wih, "whhbf": whhbf,
            "wf8": wf8, "w1": w1, "w2": w2, "gam": gamma.astype(BF),
            "bet": beta.astype(BF), "bsum": bsum, "b1": b1_pm, "b2": b2_pm,
        })

    res = bass_utils.run_bass_kernel_spmd(
        nc, in_maps, core_ids=list(range(NC_)),
        trace=bool(int(__import__("os").environ.get("KTRACE", "0"))),
    )
    _CACHE["last_result"] = res
    out = np.concatenate(
        [
            res.results[i]["mu"].reshape(A, T, BS).transpose(2, 1, 0)
            for i in range(NC_)
        ],
        axis=0,
    )
    return out


# revision 3
# speedup vs baseline: 1.0122x; 1.0122x over previous
"""Trainium2 Bass kernel v2 for nn_LSTMActor.

vs baseline:
- trunk: b_trunk folded into a padded contraction row (kills LN add); obsT on
  gpsimd queue, wtr alternating sync/scalar queues; LSTM weights prefetched
  right after the trunk DMA stream.
- LSTM: hybrid gates GEMM: k-tiles 0..3 bf16 (W x512), k-tiles 4..7 as 2 fp8
  DoubleRow pairs (W x32 fp8, h x16 fp8 -> x512). preT stored x512; Pool
  preloads it into PSUM; matmuls accumulate (start=False). Activations batched
  per j-pair from PSUM with fused 1/512 scale; elementwise split DVE/Pool.
- Head: W1 bf16 per step off bf16 h; W2 transposed (out [6,256] per step,
  bias+tanh fused on ACT into muT).
"""

import numpy as np
import ml_dtypes

import concourse.bass as bass
import concourse.tile as tile
from concourse import mybir, bacc
from concourse import bass_utils
from concourse.masks import make_identity

BF = ml_dtypes.bfloat16
E4 = ml_dtypes.float8_e4m3
F32 = mybir.dt.float32
BF16 = mybir.dt.bfloat16
FP8 = mybir.dt.float8e4
DR = mybir.MatmulPerfMode.DoubleRow

B, R, Fd, H, A, T = 2048, 39200, 1024, 1024, 6, 16
NC_ = 8
BS = B // NC_          # 256 rows per core
NB = BS // 128         # 2 b-tiles per core
KT = 128
RP = ((R + KT - 1) // KT) * KT + 128   # 39424: pad + bias k-tile
NK = RP // KT          # 308 trunk K-tiles
KH = H // 128          # 8 k-tiles for H-dim GEMMs
KBF = 4                # gates k-tiles 0..3 in bf16
M4 = 4 * H // 128      # 32 m-tiles of gates
H2 = H // 2
KG = 2                 # trunk K-tiles per DMA batch
SCL = 512.0            # gates psum scale
SH, SW8 = 16.0, 32.0   # h-fp8 scale, W-fp8 scale (SH*SW8 == SCL)

_CACHE = {}


def _build():
    nc = bacc.Bacc("TRN2", target_bir_lowering=False, debug=False)

    obsT_d = nc.dram_tensor("obsT", [RP, BS], BF16, kind="ExternalInput")
    wtr_d = nc.dram_tensor("wtr", [RP, Fd], BF16, kind="ExternalInput")
    wih_d = nc.dram_tensor("wih", [M4, 128, KH * 128], BF16, kind="ExternalInput")
    whhbf_d = nc.dram_tensor("whhbf", [128, KBF, 4 * H], BF16, kind="ExternalInput")
    wf8_d = nc.dram_tensor("wf8", [128, 2 * M4 * 2 * 128], FP8, kind="ExternalInput")
    w1_d = nc.dram_tensor("w1", [H, H2], BF16, kind="ExternalInput")
    w2_d = nc.dram_tensor("w2", [H2, A], BF16, kind="ExternalInput")
    gam_d = nc.dram_tensor("gam", [Fd], BF16, kind="ExternalInput")
    bet_d = nc.dram_tensor("bet", [Fd], BF16, kind="ExternalInput")
    bsum_d = nc.dram_tensor("bsum", [128, M4], F32, kind="ExternalInput")  # x512
    b1_d = nc.dram_tensor("b1", [128, H2 // 128], F32, kind="ExternalInput")
    b2_d = nc.dram_tensor("b2", [A, 1], F32, kind="ExternalInput")
    mu_d = nc.dram_tensor("mu", [A, T * BS], F32, kind="ExternalOutput")

    AF = mybir.ActivationFunctionType
    ALU = mybir.AluOpType

    def bc(ap1d, p=128):
        return bass.AP(tensor=ap1d.tensor, offset=ap1d.offset,
                       ap=[[0, p]] + [list(x) for x in ap1d.ap])

    with tile.TileContext(nc) as tc:
        with (
            tc.tile_pool(name="const", bufs=1) as cst,
            tc.tile_pool(name="state", bufs=1) as st,
            tc.tile_pool(name="wstream", bufs=2) as ws,
            tc.tile_pool(name="work", bufs=1) as wk,
            tc.tile_pool(name="acts", bufs=2) as ac,
        ):
            # ---- resident constants ----
            ident = cst.tile([128, 128], BF16)
            make_identity(nc, ident)
            whhbf = cst.tile([128, KBF, 4 * H], BF16)     # 32KB/part (x512)
            wf8 = cst.tile([128, 2, M4, 2, 128], FP8)     # 16KB/part (x32)
            w1_sb = cst.tile([128, KH, H2], BF16)         # 8KB/part
            w2_sb = cst.tile([128, H2 // 128, A], BF16)
            gam_b = cst.tile([128, Fd], BF16)
            bet_b = cst.tile([128, Fd], BF16)
            bsum_sb = cst.tile([128, M4], F32)            # x512 host-side
            b1_sb = cst.tile([128, H2 // 128], F32)
            b2_sb = cst.tile([A, 1], F32)
            eps_t = cst.tile([128, 1], F32)
            nc.vector.memset(eps_t, 1e-5)

            # ---- persistent state ----
            xT = st.tile([128, KH, BS], BF16)
            preT = st.tile([128, 4, 4, 2, BS], BF16)      # x512 [p,jj,gate,j01,b]
            c_st = st.tile([128, KH, BS], F32)
            h_bf = [st.tile([128, KH, BS], BF16, name=f"hbf{i}") for i in range(2)]
            h8 = [st.tile([128, 2, 2, BS], FP8, name=f"h8_{i}") for i in range(2)]
            relu1 = [st.tile([128, H2 // 128, BS], BF16, name=f"r1_{i}")
                     for i in range(2)]
            muT = st.tile([A, T, BS], F32)

            wtr_r = wtr_d.ap().rearrange("(ko p) n -> p ko n", p=128)
            obsT_r = obsT_d.ap().rearrange("(ko p) b -> p ko b", p=128)

            # ================= Phase 1: trunk GEMM =================
            front = tc.tile_pool(name="front", bufs=1)
            fr = front.__enter__()
            wihA = fr.tile([128, 8, KH * 128], BF16)      # 16KB/part, m 0..7
            with tc.tile_pool(name="ps_trunk", bufs=1, space="PSUM") as pst:
                psx = pst.tile([128, NB, Fd], F32)        # 8 banks
                for kg in range(0, NK, KG):
                    kn = min(KG, NK - kg)
                    wt = ws.tile([128, KG, Fd], BF16, tag="wtr", bufs=6)
                    ot = ws.tile([128, KG, BS], BF16, tag="obsT", bufs=4)
                    nc.sync.dma_start(wt[:, :kn, :], wtr_r[:, kg : kg + kn, :])
                    nc.sync.dma_start(ot[:, :kn, :], obsT_r[:, kg : kg + kn, :])
                    if kg == 0:
                        # small consts early (needed right at LN)
                        nc.sync.dma_start(gam_b, bc(gam_d.ap()))
                        nc.sync.dma_start(bet_b, bc(bet_d.ap()))
                        nc.sync.dma_start(bsum_sb, bsum_d.ap())
                        nc.sync.dma_start(b1_sb, b1_d.ap())
                        nc.sync.dma_start(b2_sb, b2_d.ap())
                    for kk in range(kn):
                        k = kg + kk
                        for b in range(NB):
                            lhsT = ot[:, kk, b * 128 : (b + 1) * 128]
                            for n in range(2):
                                nc.tensor.matmul(
                                    psx[:, b, n * 512 : (n + 1) * 512],
                                    lhsT,
                                    wt[:, kk, n * 512 : (n + 1) * 512],
                                    start=(k == 0),
                                    stop=(k == NK - 1),
                                )
                # weight prefetch behind the trunk stream on the same queue
                for m in range(8):
                    nc.sync.dma_start(wihA[:, m, :], wih_d.ap()[m])
                nc.sync.dma_start(whhbf, whhbf_d.ap())
                nc.sync.dma_start(
                    wf8.rearrange("p a b c d -> p (a b c d)"), wf8_d.ap())
                nc.sync.dma_start(
                    w1_sb, w1_d.ap().rearrange("(ko p) n -> p ko n", p=128))
                nc.sync.dma_start(
                    w2_sb, w2_d.ap().rearrange("(ko p) n -> p ko n", p=128))

                # ============ Phase 2: LayerNorm + tanh ============
                xa = fr.tile([128, NB, Fd], BF16, tag="xa")
                for b in range(NB):
                    stats = wk.tile([128, 2, 6], F32, tag="stats")
                    for s in range(2):
                        nc.vector.bn_stats(
                            out=stats[:, s, :],
                            in_=psx[:, b, s * 512 : (s + 1) * 512])
                    mv = wk.tile([128, 2], F32, tag="mv")
                    nc.vector.bn_aggr(out=mv, in_=stats)
                    rstd = wk.tile([128, 1], F32, tag="rstd")
                    nc.scalar.activation(
                        out=rstd, in_=mv[:, 1:2], func=AF.Sqrt, bias=eps_t,
                        scale=1.0)
                    nc.vector.reciprocal(out=rstd, in_=rstd)
                    xs = wk.tile([128, Fd], F32, tag="xs", bufs=1)
                    nc.vector.scalar_tensor_tensor(
                        out=xs, in0=psx[:, b, :], scalar=mv[:, 0:1], in1=gam_b,
                        op0=ALU.subtract, op1=ALU.mult)
                    nc.vector.scalar_tensor_tensor(
                        out=xs, in0=xs, scalar=rstd, in1=bet_b,
                        op0=ALU.mult, op1=ALU.add)
                    nc.scalar.activation(out=xa[:, b, :], in_=xs, func=AF.Tanh)

            # ============ Phase 3: transpose x -> xT ============
            with tc.tile_pool(name="ps_tr", bufs=4, space="PSUM") as ptr:
                for b in range(NB):
                    for f in range(KH):
                        pt = ptr.tile([128, 128], BF16, tag="tr")
                        nc.tensor.transpose(
                            pt, xa[:, b, f * 128 : (f + 1) * 128], ident)
                        nc.scalar.activation(
                            out=xT[:, f, b * 128 : (b + 1) * 128], in_=pt,
                            func=AF.Copy)

            # ============ Phase 4: preT = 512*(W_ih^T x^T) + 512*bias ======
            with tc.tile_pool(name="ps_pre", bufs=4, space="PSUM") as ppr:
                for m in range(M4):
                    if m < 8:
                        wm = wihA[:, m, :].rearrange("p (k j) -> p k j", j=128)
                    else:
                        wmt = ws.tile([128, KH, 128], BF16, tag="wih", bufs=4)
                        nc.sync.dma_start(
                            wmt, wih_d.ap()[m].rearrange("p (k j) -> p k j", j=128))
                        wm = wmt
                    ps = ppr.tile([128, BS], F32, tag="pre")
                    for k in range(KH):
                        nc.tensor.matmul(
                            ps, wm[:, k, :], xT[:, k, :],
                            start=(k == 0), stop=(k == KH - 1))
                    g, j = m // 8, m % 8
                    nc.scalar.activation(
                        out=preT[:, j // 2, g, j % 2, :], in_=ps, func=AF.Identity,
                        bias=bsum_sb[:, m : m + 1], scale=SCL)

            front.__exit__(None, None, None)

            # ============ Phase 5: LSTM ============
            with tc.tile_pool(name="ps_g", bufs=2, space="PSUM") as psg:

                def elementwise(blk, jj, t, first):
                    j0 = 2 * jj
                    cur = t % 2
                    if first:
                        src = preT[:, jj]
                    else:
                        # gates = psum + preT (DVE) in two halves so the
                        # sigmoid(i,f) can start while (g,o) still adds
                        src = ac.tile([128, 4, 2, BS], F32, tag="gad")
                        nc.vector.tensor_add(src[:, 0:2], blk[:, 0:2],
                                             preT[:, jj, 0:2])
                        nc.vector.tensor_add(src[:, 2:4], blk[:, 2:4],
                                             preT[:, jj, 2:4])
                    scl = 1.0 / SCL
                    sif = ac.tile([128, 2, 2, BS], BF16, tag="sif")
                    nc.scalar.activation(out=sif, in_=src[:, 0:2],
                                         func=AF.Sigmoid, scale=scl)
                    tg = ac.tile([128, 2, BS], BF16, tag="tg")
                    nc.scalar.activation(out=tg, in_=src[:, 2], func=AF.Tanh,
                                         scale=scl)
                    so = ac.tile([128, 2, BS], BF16, tag="so")
                    nc.scalar.activation(out=so, in_=src[:, 3], func=AF.Sigmoid,
                                         scale=scl)
                    cs = c_st[:, j0 : j0 + 2, :]
                    if first:
                        nc.vector.tensor_mul(cs, sif[:, 0], tg)
                    else:
                        t1 = ac.tile([128, 2, BS], BF16, tag="t1")
                        nc.vector.tensor_mul(t1, sif[:, 0], tg)
                        nc.vector.tensor_mul(cs, cs, sif[:, 1])
                        nc.vector.tensor_add(cs, cs, t1)
                    tcn = ac.tile([128, 2, BS], BF16, tag="tc")
                    nc.scalar.activation(out=tcn, in_=cs, func=AF.Tanh)
                    nc.vector.tensor_mul(h_bf[cur][:, j0 : j0 + 2, :], so, tcn)
                    if jj >= 2:
                        nc.vector.scalar_tensor_tensor(
                            out=h8[cur][:, jj - 2], in0=so, scalar=SH, in1=tcn,
                            op0=ALU.mult, op1=ALU.mult)

                def head(t):
                    """relu1(t) = relu(W1^T h(t) + b1)."""
                    cur = t % 2
                    blk = psg.tile([128, 4, 2, BS], F32, tag="g")
                    for m2 in range(H2 // 128):
                        for k in range(KH):
                            nc.tensor.matmul(
                                blk[:, m2 // 2, m2 % 2, :],
                                w1_sb[:, k, m2 * 128 : (m2 + 1) * 128],
                                h_bf[cur][:, k, :],
                                start=(k == 0), stop=(k == KH - 1))
                    for m2 in range(H2 // 128):
                        nc.scalar.activation(
                            out=relu1[cur][:, m2, :],
                            in_=blk[:, m2 // 2, m2 % 2, :],
                            func=AF.Relu, bias=b1_sb[:, m2 : m2 + 1], scale=1.0)

                def w2_head(t):
                    """muT[:, t, :] = tanh(W2^T relu1(t) + b2)."""
                    cur = t % 2
                    blk = psg.tile([128, 4, 2, BS], F32, tag="g")
                    ps2 = blk[:A, 0, 0, :]
                    for k2 in range(H2 // 128):
                        nc.tensor.matmul(
                            ps2, w2_sb[:, k2, :], relu1[cur][:, k2, :],
                            start=(k2 == 0), stop=(k2 == H2 // 128 - 1))
                    nc.scalar.activation(
                        out=muT[:, t, :], in_=ps2, func=AF.Tanh,
                        bias=b2_sb, scale=1.0)

                # ---- step 0: gates = pre ----
                for jj in range(4):
                    elementwise(None, jj, 0, first=True)

                # ---- steps 1..15 ----
                for t in range(1, T):
                    prev = (t + 1) % 2
                    head(t - 1)
                    for jj in range(4):
                        blk = psg.tile([128, 4, 2, BS], F32, tag="g")
                        # one open accumulation group per PSUM bank: open the
                        # 4 bank-distinct regions (one per gate) with bf16,
                        # then close each with its fp8 DR pair
                        for j01 in range(2):
                            for g in range(4):
                                m = g * 8 + 2 * jj + j01
                                for k in range(KBF):
                                    nc.tensor.matmul(
                                        blk[:, g, j01, :],
                                        whhbf[:, k, m * 128 : (m + 1) * 128],
                                        h_bf[prev][:, k, :],
                                        start=(k == 0), stop=False,
                                        skip_group_check=True)
                            for g in range(4):
                                m = g * 8 + 2 * jj + j01
                                for p in range(2):
                                    nc.tensor.matmul(
                                        blk[:, g, j01, :],
                                        wf8[:, p, m, :, :],
                                        h8[prev][:, p],
                                        start=False, stop=(p == 1),
                                        perf_mode=DR,
                                        skip_group_check=True)
                        elementwise(blk, jj, t, first=False)
                    if t >= 2:
                        w2_head(t - 2)
                head(T - 1)
                w2_head(T - 2)
                w2_head(T - 1)

            # ---- write out ----
            nc.sync.dma_start(
                mu_d.ap().rearrange("a (t b) -> a t b", t=T), muT)

    nc.compile()
    return nc


def kernel(**inputs):
    obs = np.asarray(inputs["obs"], np.float32)
    W_trunk = np.asarray(inputs["W_trunk"], np.float32)
    b_trunk = np.asarray(inputs["b_trunk"], np.float32)
    gamma = np.asarray(inputs["gamma"], np.float32)
    beta = np.asarray(inputs["beta"], np.float32)
    W_ih = np.asarray(inputs["W_ih"], np.float32)
    b_ih = np.asarray(inputs["b_ih"], np.float32)
    W_hh = np.asarray(inputs["W_hh"], np.float32)
    b_hh = np.asarray(inputs["b_hh"], np.float32)
    W1 = np.asarray(inputs["W1"], np.float32)
    b1 = np.asarray(inputs["b1"], np.float32)
    W2 = np.asarray(inputs["W2"], np.float32)
    b2 = np.asarray(inputs["b2"], np.float32)
    num_actions = int(np.asarray(inputs["num_actions"]))
    assert num_actions == T, f"kernel hardcodes T={T}, got {num_actions}"
    assert obs.shape == (B, R)

    if "nc" not in _CACHE:
        _CACHE["nc"] = _build()
    nc = _CACHE["nc"]

    wtr = np.zeros((RP, Fd), BF)
    wtr[:R] = W_trunk.astype(BF)
    wtr[R] = b_trunk.astype(BF)          # bias contraction row
    wih = np.ascontiguousarray(
        W_ih.astype(BF).reshape(KH, 128, M4, 128).transpose(2, 1, 0, 3)
    ).reshape(M4, 128, KH * 128)
    whhbf = np.ascontiguousarray(
        (W_hh[: KBF * 128] * SCL).astype(BF).reshape(KBF, 128, 4 * H)
        .transpose(1, 0, 2))
    wf8 = np.ascontiguousarray(
        (W_hh[KBF * 128 :] * SW8).astype(E4)
        .reshape(2, 2, 128, M4, 128).transpose(2, 0, 3, 1, 4)).reshape(128, -1)
    w1 = W1.astype(BF)
    w2 = W2.astype(BF)
    bsum = np.ascontiguousarray(
        ((b_ih + b_hh) * SCL).astype(np.float32).reshape(M4, 128).T)
    b1_pm = np.ascontiguousarray(b1.astype(np.float32).reshape(H2 // 128, 128).T)
    b2_pm = b2.astype(np.float32).reshape(A, 1)

    in_maps = []
    for i in range(NC_):
        sh = obs[i * BS : (i + 1) * BS]           # [256, R]
        obsT = np.zeros((RP, BS), BF)
        obsT[:R] = np.ascontiguousarray(sh.T).astype(BF)
        obsT[R] = 1.0                              # bias row
        in_maps.append({
            "obsT": obsT, "wtr": wtr, "wih": wih, "whhbf": whhbf,
            "wf8": wf8, "w1": w1, "w2": w2, "gam": gamma.astype(BF),
            "bet": beta.astype(BF), "bsum": bsum, "b1": b1_pm, "b2": b2_pm,
        })

    res = bass_utils.run_bass_kernel_spmd(
        nc, in_maps, core_ids=list(range(NC_)),
        trace=bool(int(__import__("os").environ.get("KTRACE", "0"))),
    )
    _CACHE["last_result"] = res
    out = np.concatenate(
        [
            res.results[i]["mu"].reshape(A, T, BS).transpose(2, 1, 0)
            for i in range(NC_)
        ],
        axis=0,
    )
    return out
